# revision 8
# baseline (speedup 1.0000x reference)
"""CornerNet Trainium2 kernel — low-rank functional expansion.

Math (reference):
  t     = kappa * tanh(sign_param) * (x - th)        # (B, R, D)
  s     = sigmoid(t); m = sigmoid(mask_logit)
  gated = 1 - m*(1-s)
  z     = prod_d gated                               # (B, R)
  y     = z @ head_w.T + head_b                      # (B,)

FAST PATH (uniform mask_logit — the actual model):
  log z[b,r] = sum_d phi(a[r,d]*x[b,d] - b[r,d]),
  phi(t) = ln(1 - m + m*sigmoid(t)),  a = kappa*tanh(sign_param), b = a*th.

  phi(a*x - b) is approximated per (r,d) element by a weighted least-squares
  fit onto a small dictionary of functions of x alone:
      { 1, x, phi(alpha_1 x), ..., phi(alpha_NPHI x) }
  so  log z = C_const (summed over d, folded into the final Exp bias)
            + sum_{k in basis} F_k @ G_k(x),
  i.e. 1+NPHI matmuls on TensorE instead of B*R*D scalar-engine work (y rel
  err ~1e-3 at NPHI=3, fit-residual-checked on the host per call).  The
  dictionary coefficients are computed exactly per element on the host
  (adapting to the actual kappa/th/sign_param/x distribution), and
  phi(alpha x) is ONE ScalarE pass per alpha via the gelu activation-table
  slot re-fitted to phi_m (the NEFF embeds the tables;
  BASS_ACT_ROOT_JSON_PATH selects them).  The scales were optimized by
  multi-start coordinate descent on the family fit residual, which also
  keeps the coefficients small (max|C| ~4) so table-approximation error is
  not amplified.  If the inputs are outside what the dictionary can
  represent the kernel falls back to the exact per-rule path below.

  Sharding: 2 rule-groups x 4 batch-groups over 8 cores.  Per core: 256
  rules (2 PSUM tiles of 128) x 512 batch, D=256 on partitions in 2 halves.
  Per rep per core: NPHI ScalarE activation passes + 4*(1+NPHI) f32r
  matmuls (512 rows each) + final Exp (bias = constant term) and a tiny
  head matmul.  At NPHI=3: TensorE ~3.4us and ScalarE ~3.1us per rep —
  both engines near their streaming floors.

FALLBACK (non-uniform mask_logit or poor fit): per-rule sigmoid/ln path,
tensor-parallel over rules (8x64), ScalarE-bound (~343us).
"""

import numpy as np
import math
from contextlib import ExitStack

import ml_dtypes

import concourse.bass as bass
import concourse.bacc as bacc
import concourse.mybir as mybir
import concourse.tile as tile
from concourse.bass_utils import run_bass_kernel_spmd
from bass_rust import add_dep_helper

B, D, R = 2048, 256, 512
NCORES = 8
F32 = mybir.dt.float32
F32R = mybir.dt.float32r
FP8 = mybir.dt.float8e4
NP8 = ml_dtypes.float8_e4m3
AF = mybir.ActivationFunctionType
OP = mybir.AluOpType

# ---- low-rank fast path geometry ----
GR, GB = 2, 4            # rule groups x batch groups
RS = R // GR             # 256 rules per core (2 tiles of 128)
BS = B // GB             # 512 batch per core
POWERS = ()              # extra monomial basis (computed on DVE)
NPHI = 3
NB = 1 + len(POWERS) + NPHI   # matmul basis: x, powers, phi(alpha_k x)
# multi-start coordinate-polished scale ratios (relative to max|a|); the
# polish also shrinks the fit coefficients (max|C| ~4 vs ~180), which keeps
# the activation-table approximation error of phi from being amplified
RATIOS = (0.3177, -0.3388, 0.6539)

_cache = {}


# ======================================================================
# Activation tables: refit the `gelu` spline buckets to phi_m.
# ======================================================================

import hashlib
import json
import os
import shutil
import tempfile

TABLE_VERSION = "v2"


def _phi64(u, m):
    c = 1.0 - m
    u = np.asarray(u, np.float64)
    return np.logaddexp(np.log(c), u) - np.logaddexp(0.0, u)


def _fit_cubic(lo, hi, x0, m):
    u = np.linspace(lo, hi, 129)
    y = _phi64(u, m)
    A = np.vander(u - x0, 4, increasing=True)
    coef, *_ = np.linalg.lstsq(A, y, rcond=None)
    return coef


def _patch_gelu_tables(dstdir, m, hi):
    jpath = os.path.join(dstdir, "gelu_and_others.json")
    d = json.load(open(jpath))
    cnt = d["bkt_entry_cnt"]
    bpath = os.path.join(dstdir, "gelu_and_others_bkt.bin")
    bkt = np.fromfile(bpath, dtype=np.float32).reshape(cnt, 8).copy()

    fx = d["func_exp_to_bkt_start_idx"]["gelu"]
    negs = sorted([(int(e), v[0]) for e, v in fx.items()], key=lambda t: t[1])
    poss = sorted([(int(e), v[1]) for e, v in fx.items() if len(v) > 1],
                  key=lambda t: t[1])
    neg_bounds = [s for _, s in negs] + [poss[0][1]]
    pos_bounds = [s for _, s in poss] + [504]

    for side, lst, bounds in (("neg", negs, neg_bounds), ("pos", poss, pos_bounds)):
        for i, (e, start) in enumerate(lst):
            n = bounds[i + 1] - start
            x0s = bkt[start : start + n, 4].astype(np.float64)
            if n >= 2:
                w = abs(x0s[1] - x0s[0])
            else:
                w = 2.0 ** e
            for j in range(n):
                x0 = float(x0s[j])
                lo, hi_ = x0 - w / 2, x0 + w / 2
                bkt[start + j, 0:4] = _fit_cubic(lo, hi_, x0, m).astype(np.float32)
    # special buckets: small-signal (|u|<2^-7) and large-signal tails.
    # gelu profile thresholds: pos-large 4.918, neg-large -8.374.  The tails
    # must cover max|alpha*x| (phi is ~flat there so one cubic is plenty).
    for k, (lo, hi_, x0) in {
        504: (1e-7, 2.0 ** -7, 0.0),
        505: (-(2.0 ** -7), -1e-7, 0.0),
        506: (4.918, hi, (4.918 + hi) / 2),
        507: (-hi, -8.374, -(hi + 8.374) / 2),
    }.items():
        bkt[k, 0:4] = _fit_cubic(lo, hi_, x0, m).astype(np.float32)
        bkt[k, 4] = x0
    bkt.tofile(bpath)

    def f32bits(v):
        return int(np.float32(v).view(np.uint32))

    for pm in d["profile_meta_data"]:
        if pm["func_name"].startswith("gelu_"):
            pm["fzero_result"] = f32bits(_phi64(0.0, m))
            pm["fpinf_result"] = 0
            pm["fninf_result"] = f32bits(np.log(1.0 - m))
    with open(jpath, "w") as f:
        json.dump(d, f)


def _gen_act_tables(m, hi):
    """Build a patched act-table dir (gelu := phi_m); returns (json_path, tag)."""
    from neuronxcc.driver.Job import Job
    from neuronxcc.driver.jobs.support.FindActInfo import findActInfoFile

    src_json = findActInfoFile(Job.getPackageDir(), "gen3")
    srcdir = os.path.dirname(src_json)
    tag = hashlib.md5(
        (TABLE_VERSION + repr(float(np.float64(m))) + repr(float(hi))).encode()
    ).hexdigest()[:10]
    dstdir = os.path.join(tempfile.gettempdir(), f"cn_act_{tag}")
    marker = os.path.join(dstdir, "act_info.json")
    if not os.path.isfile(marker):
        tmp = dstdir + ".tmp"
        shutil.rmtree(tmp, ignore_errors=True)
        os.makedirs(tmp)
        for f in os.listdir(srcdir):
            shutil.copyfile(os.path.join(srcdir, f), os.path.join(tmp, f))
        _patch_gelu_tables(tmp, m, hi)
        shutil.rmtree(dstdir, ignore_errors=True)
        try:
            os.rename(tmp, dstdir)
        except OSError:
            if not os.path.isfile(marker):
                raise
    return marker, tag


# ======================================================================
# Low-rank kernel build
# ======================================================================


def _build_lr(reps, alphas, tag):
    nc = bacc.Bacc(None)
    xs = nc.dram_tensor("xs", [128, 2 * BS], F32R, kind="ExternalInput")
    fpk = nc.dram_tensor(f"fpk_{tag}", [128, NB * 4 * 128], F32R,
                         kind="ExternalInput")
    eb = nc.dram_tensor("eb", [128, GR], F32, kind="ExternalInput")
    whd = nc.dram_tensor("whd", [128, GR], F32R, kind="ExternalInput")
    y = nc.dram_tensor("y", [1, BS], F32, kind="ExternalOutput")

    with tile.TileContext(nc) as tc, ExitStack() as ctx:
        const = ctx.enter_context(tc.tile_pool(name="const", bufs=1))
        gp = ctx.enter_context(tc.tile_pool(name="gp", bufs=4))
        pp = ctx.enter_context(tc.tile_pool(name="pp", bufs=2))
        psum = ctx.enter_context(
            tc.tile_pool(name="psum", bufs=1, space=bass.MemorySpace.PSUM)
        )

        xs_t = const.tile([128, 2 * BS], F32R, tag="xs")
        nc.gpsimd.dma_start(xs_t[:], xs[:])
        fpk_t = const.tile([128, NB * 4 * 128], F32R, tag="fpk")
        for q in range(4):
            w = NB * 128
            nc.gpsimd.dma_start(fpk_t[:, q * w : (q + 1) * w],
                                fpk[:, q * w : (q + 1) * w])
        eb_t = const.tile([128, GR], F32, tag="eb")
        nc.gpsimd.dma_start(eb_t[:], eb[:])
        whd_t = const.tile([128, GR], F32R, tag="whd")
        nc.gpsimd.dma_start(whd_t[:], whd[:])

        lz0 = psum.tile([128, BS], F32, tag="lz0")
        lz1 = psum.tile([128, BS], F32, tag="lz1")
        lz = [lz0, lz1]

        npow = len(POWERS)
        nmm = reps * NB * 2          # matmuls per lz tile (halves x basis x reps)
        imm = 0
        for rep in range(reps):
            rhs_tiles = {0: xs_t}
            prev = xs_t
            for pi in range(npow):
                pw_t = pp.tile([128, 2 * BS], F32R, tag=f"pw{pi}")
                nc.vector.tensor_mul(pw_t[:], prev[:], xs_t[:])
                rhs_tiles[1 + pi] = pw_t
                prev = pw_t
            for k in range(NPHI):
                g = gp.tile([128, 2 * BS], F32R, tag="g")
                nc.scalar.activation(g[:], xs_t[:], AF.Gelu, scale=alphas[k])
                rhs_tiles[1 + npow + k] = g
            for bi in range(NB):
                rhs = rhs_tiles[bi]
                for h in range(2):
                    for rt in range(GR):
                        nc.tensor.matmul(
                            lz[rt][:, :],
                            fpk_t[:, (bi * 4 + h * 2 + rt) * 128 :
                                  (bi * 4 + h * 2 + rt + 1) * 128],
                            rhs[:, h * BS : (h + 1) * BS],
                            start=(imm // 2 == 0),
                            stop=(imm // 2 == nmm - 1),
                        )
                        imm += 1

        yp = psum.tile([1, BS], F32, tag="yp")
        for rt in range(GR):
            z_t = const.tile([128, BS], F32R, tag=f"z{rt}")
            nc.scalar.activation(z_t[:], lz[rt][:], AF.Exp,
                                 bias=eb_t[:, rt : rt + 1])
            nc.tensor.matmul(yp[:, :], whd_t[:, rt : rt + 1], z_t[:],
                             start=(rt == 0), stop=(rt == GR - 1))
        y_sb = const.tile([1, BS], F32, tag="ysb")
        nc.vector.tensor_copy(y_sb[:], yp[:])
        nc.sync.dma_start(y[:], y_sb[:])

    nc.compile()
    return nc


def _get_nc_lr(reps, alphas, tag):
    key = ("lr", reps, alphas, tag)
    if key not in _cache:
        _cache[key] = _build_lr(reps, alphas, tag)
    return _cache[key]


# ======================================================================
# Host-side fit + packing
# ======================================================================


def _mask_const(inputs):
    mk = np.asarray(inputs["mask_logit"], dtype=np.float64)
    v = mk.reshape(-1)[0]
    return float(v) if np.all(mk == v) else None


def _prep_lr(inputs):
    """Fit dictionary coefficients; returns (in_maps, alphas, tag, hb) or
    None if the fast path doesn't apply."""
    mkv = _mask_const(inputs)
    if mkv is None:
        return None
    m = 1.0 / (1.0 + np.exp(-np.float64(mkv)))
    if not (1e-8 < m < 1 - 1e-8):
        return None

    x = np.asarray(inputs["x"], dtype=np.float64)
    sg = np.asarray(inputs["sign_param"], dtype=np.float64)
    th = np.asarray(inputs["th"], dtype=np.float64)
    lk = float(np.asarray(inputs["log_kappa"], dtype=np.float64).reshape(-1)[0])
    hwt = np.asarray(inputs["head_w"], dtype=np.float64).reshape(-1)
    hb = float(np.asarray(inputs["head_b"], dtype=np.float64).reshape(-1)[0])

    kappa = np.exp(lk)
    a = kappa * np.tanh(sg)                 # (R, D)
    bb = a * th                             # (R, D) per-element bias
    amax = float(np.abs(a).max())
    xmax = float(np.abs(x).max())
    if amax == 0.0:
        return None
    alphas = tuple(round(r * amax, 4) for r in RATIOS)
    hi = float(max(12.0, np.ceil(1.10 * max(abs(al) for al in alphas) * xmax)))

    # ---- fit grid: empirical x-weights ----
    L = max(5.6, 1.05 * xmax)
    NX = 1601
    xg = np.linspace(-L, L, NX)
    hcnt, _ = np.histogram(x.reshape(-1), bins=NX,
                           range=(-L - L / (NX - 1) / 2, L + L / (NX - 1) / 2))
    wx = hcnt.astype(np.float64) + hcnt.max() * 1e-3
    wx /= wx.sum()

    cols = [np.ones_like(xg), xg] + [xg**p for p in POWERS]
    for al in alphas:
        cols.append(_phi64(al * xg, m))
    Dm = np.stack(cols, 1)                  # (NX, S)  S = 1 + NB
    S = Dm.shape[1]
    G = (Dm * wx[:, None]).T @ Dm
    G += 1e-13 * np.trace(G) * np.eye(S) / S
    K = np.linalg.solve(G, (Dm * wx[:, None]).T)   # (S, NX)

    af = a.reshape(-1)
    bf = bb.reshape(-1)
    C = np.empty((af.size, S))
    res2 = np.empty(af.size)
    for i0 in range(0, af.size, 8192):
        asl = af[i0:i0 + 8192]
        bsl = bf[i0:i0 + 8192]
        P = _phi64(asl[:, None] * xg[None, :] - bsl[:, None], m)
        Cc = P @ K.T
        C[i0:i0 + 8192] = Cc
        Rm = P - Cc @ Dm.T
        res2[i0:i0 + 8192] = (Rm * Rm) @ wx
    # per-rule predicted logz error std: sqrt(sum_d res2)
    err_r = np.sqrt(res2.reshape(R, D).sum(axis=1))
    if err_r.max() > 2.5e-3:
        return None

    C = C.reshape(R, D, S)

    # ---- pack per-core operands ----
    xT = x.T                                 # (D, B)
    in_maps = []
    for c in range(NCORES):
        gr, gb = c // GB, c % GB
        bsl = slice(gb * BS, (gb + 1) * BS)
        xs_arr = np.concatenate([xT[0:128, bsl], xT[128:256, bsl]], axis=1)
        sub = C[gr * RS : (gr + 1) * RS]     # (RS, D, S)
        # [rt, j, h, p, s] -> [p, s-1(bi), h, rt, j]
        t5 = sub.reshape(GR, 128, 2, 128, S).transpose(3, 4, 2, 0, 1)
        fpk_arr = np.ascontiguousarray(
            t5[:, 1:, :, :, :].reshape(128, NB * 4 * 128), dtype=np.float32)
        eb_arr = np.ascontiguousarray(
            sub[:, :, 0].sum(axis=1).reshape(GR, 128).T, dtype=np.float32)
        whd_arr = np.ascontiguousarray(
            hwt[gr * RS : (gr + 1) * RS].reshape(GR, 128).T, dtype=np.float32)
        m_ = {
            "xs": np.ascontiguousarray(xs_arr, dtype=np.float32),
            "eb": eb_arr,
            "whd": whd_arr,
            "__fpk": fpk_arr,
        }
        in_maps.append(m_)
    json_path, tag = _gen_act_tables(m, hi)
    for m_ in in_maps:
        m_[f"fpk_{tag}"] = m_.pop("__fpk")
    os.environ["BASS_ACT_ROOT_JSON_PATH"] = json_path
    return in_maps, alphas, tag, hb


# ======================================================================
# Generic fallback (per-rule sigmoid/ln path) — unchanged baseline
# ======================================================================

RC = R // NCORES
KBLK = 8
CH = 512


def _build(reps=1):
    nc = bacc.Bacc(None)
    xT = nc.dram_tensor("xT", [D, B], F32, kind="ExternalInput")
    thT = nc.dram_tensor("thT", [D, RC], F32, kind="ExternalInput")
    sgT = nc.dram_tensor("sgT", [D, RC], F32, kind="ExternalInput")
    mkT = nc.dram_tensor("mkT", [D, RC], F32, kind="ExternalInput")
    lkb = nc.dram_tensor("lkb", [128, 1], F32, kind="ExternalInput")
    wcol = nc.dram_tensor("wcol", [RC, 1], F32, kind="ExternalInput")
    selp = nc.dram_tensor("selp", [128, 2 * RC], F32R, kind="ExternalInput")
    y = nc.dram_tensor("y", [1, B], F32, kind="ExternalOutput")

    with tile.TileContext(nc) as tc, ExitStack() as ctx:
        const = ctx.enter_context(tc.tile_pool(name="const", bufs=1))
        sp = ctx.enter_context(tc.tile_pool(name="sp", bufs=2))
        gp_ = ctx.enter_context(tc.tile_pool(name="gp_", bufs=2))
        gpp = ctx.enter_context(tc.tile_pool(name="gpp", bufs=KBLK + 1))
        lp = ctx.enter_context(tc.tile_pool(name="lp", bufs=2))
        psum = ctx.enter_context(
            tc.tile_pool(name="psum", bufs=1, space=bass.MemorySpace.PSUM)
        )

        xt = []
        for h in range(2):
            t_ = const.tile([128, B], F32, tag=f"xt{h}")
            nc.gpsimd.dma_start(t_[:], xT[h * 128 : (h + 1) * 128, :])
            xt.append(t_)

        tht, sgt, mkt = [], [], []
        for name, dram, lst in (("th", thT, tht), ("sg", sgT, sgt), ("mk", mkT, mkt)):
            for h in range(2):
                t_ = const.tile([128, RC], F32, tag=f"{name}{h}")
                nc.gpsimd.dma_start(t_[:], dram[h * 128 : (h + 1) * 128, :])
                lst.append(t_)

        lkt = const.tile([128, 1], F32, tag="lkt")
        nc.gpsimd.dma_start(lkt[:], lkb[:])
        selpt = const.tile([128, 2 * RC], F32R, tag="selpt")
        nc.gpsimd.dma_start(selpt[:], selp[:])
        wct = const.tile([RC, 1], F32, tag="wct")
        nc.gpsimd.dma_start(wct[:], wcol[:])

        kap = const.tile([128, 1], F32, tag="kap")
        nc.scalar.activation(kap[:], lkt[:], AF.Exp)
        nkap = const.tile([128, 1], F32, tag="nkap")
        nc.vector.tensor_scalar(nkap[:], kap[:], -1.0, None, OP.mult)

        aa, nb2, mm_, cc_ = [], [], [], []
        for h in range(2):
            tnh = const.tile([128, RC], F32, tag=f"tnh{h}")
            nc.scalar.activation(tnh[:], sgt[h][:], AF.Tanh)
            a_h = const.tile([128, RC], F32, tag=f"a{h}")
            nc.vector.tensor_scalar(a_h[:], tnh[:], kap[:], None, OP.mult)
            na_h = const.tile([128, RC], F32, tag=f"na{h}")
            nc.vector.tensor_scalar(na_h[:], tnh[:], nkap[:], None, OP.mult)
            nb2_h = const.tile([128, RC], F32, tag=f"nb2{h}")
            nc.vector.tensor_mul(nb2_h[:], na_h[:], tht[h][:])
            aa.append(a_h)
            nb2.append(nb2_h)
            m_h = const.tile([128, RC], F32, tag=f"m{h}")
            nc.scalar.activation(m_h[:], mkt[h][:], AF.Sigmoid)
            c_h = const.tile([128, RC], F32, tag=f"c{h}")
            nc.scalar.activation(c_h[:], mkt[h][:], AF.Sigmoid, scale=-1.0)
            mm_.append(m_h)
            cc_.append(c_h)

        lz = psum.tile([RC, B], F32, tag="lz")
        last_ln = None
        for rep in range(reps):
            for blk in range(RC // KBLK):
                gps = []
                sig_insts = []
                for k in range(KBLK):
                    r = blk * KBLK + k
                    s = sp.tile([128, 2 * B], F32, tag="s")
                    for h in range(2):
                        si = nc.scalar.activation(
                            s[:, h * B : (h + 1) * B],
                            xt[h][:],
                            AF.Sigmoid,
                            bias=nb2[h][:, r : r + 1],
                            scale=aa[h][:, r : r + 1],
                        )
                        if last_ln is not None:
                            add_dep_helper(si.ins, last_ln.ins, False,
                                           "act-table phase blocking")
                        sig_insts.append(si)
                    g = gp_.tile([128, 2 * B], F32, tag="g")
                    for h in range(2):
                        nc.vector.tensor_scalar(
                            g[:, h * B : (h + 1) * B],
                            s[:, h * B : (h + 1) * B],
                            mm_[h][:, r : r + 1],
                            cc_[h][:, r : r + 1],
                            OP.mult,
                            OP.add,
                        )
                    gpt = gpp.tile([128, B], F32, tag="gpt")
                    nc.vector.tensor_mul(gpt[:], g[:, 0:B], g[:, B : 2 * B])
                    gps.append(gpt)
                for k in range(KBLK):
                    r = blk * KBLK + k
                    L = lp.tile([128, B], F32R, tag="L")
                    ln_i = nc.scalar.activation(L[:], gps[k][:], AF.Ln)
                    add_dep_helper(ln_i.ins, sig_insts[-1].ins, False,
                                   "act-table phase blocking")
                    last_ln = ln_i
                    lhsp = selpt[:, RC - r : 2 * RC - r]
                    for c in range(B // CH):
                        nc.tensor.matmul(
                            lz[:, c * CH : (c + 1) * CH],
                            lhsp,
                            L[:, c * CH : (c + 1) * CH],
                            start=(r == 0 and rep == 0),
                            stop=(r == RC - 1 and rep == reps - 1),
                        )

        z_sb = const.tile([RC, B], F32, tag="z")
        nc.scalar.activation(z_sb[:], lz[:], AF.Exp)
        yp = psum.tile([1, B], F32, tag="yp")
        for c in range(B // CH):
            nc.tensor.matmul(
                yp[:, c * CH : (c + 1) * CH],
                wct[:],
                z_sb[:, c * CH : (c + 1) * CH],
                start=True,
                stop=True,
            )
        y_sb = const.tile([1, B], F32, tag="ysb")
        nc.vector.tensor_copy(y_sb[:], yp[:])
        nc.sync.dma_start(y[:], y_sb[:])

    nc.compile()
    return nc


def _get_nc(reps=1):
    key = ("nc", reps)
    if key not in _cache:
        _cache[key] = _build(reps)
    return _cache[key]


def _make_in_maps(inputs):
    x = np.ascontiguousarray(inputs["x"], dtype=np.float32)
    th = np.asarray(inputs["th"], dtype=np.float32)
    sg = np.asarray(inputs["sign_param"], dtype=np.float32)
    mk = np.asarray(inputs["mask_logit"], dtype=np.float32)
    lk = float(np.asarray(inputs["log_kappa"], dtype=np.float32).reshape(-1)[0])
    hw = np.asarray(inputs["head_w"], dtype=np.float32)

    xT = np.ascontiguousarray(x.T)
    lkb = np.full((128, 1), lk, dtype=np.float32)
    selp = np.zeros((128, 2 * RC), dtype=np.float32)
    selp[:, RC] = 1.0

    in_maps = []
    for c in range(NCORES):
        sl = slice(c * RC, (c + 1) * RC)
        in_maps.append(
            {
                "xT": xT,
                "thT": np.ascontiguousarray(th[sl].T),
                "sgT": np.ascontiguousarray(sg[sl].T),
                "mkT": np.ascontiguousarray(mk[sl].T),
                "lkb": lkb,
                "wcol": np.ascontiguousarray(hw.reshape(-1)[sl].reshape(RC, 1)),
                "selp": selp,
            }
        )
    return in_maps


# ======================================================================
# V2 path: fp8 DoubleRow + designed act-table bases (see module doc)
# ======================================================================

# ---- v2 geometry: data-parallel over batch ----
BSV = B // NCORES        # 256 batch per core
NBAS = 3                 # matmul bases: x, v1, v2
NRT = R // 128           # 4 rule tiles

# ---- v2 fit hyperparameters ----
V2_CAP = 4.0             # basis amplitude cap (fp8 noise tail control)
V2_LAM = 0.022           # ridge = fp8 C-noise rms (relative)
V2_NITER = 10            # alternating iterations per basis
V2_ASUB = 12288          # a-subsample for basis optimization
V2_GATE = 1.2e-2         # max per-rule logz residual std to accept

import hashlib
import json
import os
import shutil
import tempfile

TABLE_VERSION_V2 = "v2t1"


def _phi64(u, m):
    c = 1.0 - m
    u = np.asarray(u, np.float64)
    return np.logaddexp(np.log(c), u) - np.logaddexp(0.0, u)


def _q8(v, e):
    """fp8e4m3 (trn float8e4) quantize with scale 2**e, dequantized float."""
    return (np.asarray(v * (2.0**e), np.float32).astype(NP8).astype(np.float64)
            ) * 2.0 ** (-e)


def _scale_exp(maxabs):
    if maxabs <= 0:
        return 0
    return int(math.floor(math.log2(240.0 / maxabs)))


# ======================================================================
# Activation tables: gelu := fg, tanh := ft (ft odd; hw mirrors sign)
# ======================================================================


def _fit_cubic_f(lo, hi, x0, f):
    u = np.linspace(lo, hi, 129)
    y = f(u)
    A = np.vander(u - x0, 4, increasing=True)
    coef, *_ = np.linalg.lstsq(A, y, rcond=None)
    return coef


def _thr_from_meta(exp_thr, mant_thr):
    if exp_thr == 0:
        return None
    return 2.0 ** (exp_thr - 127) * (1.0 + mant_thr / 2.0**23)


def _patch_tables_v2(dstdir, fg, ft, hi):
    """Refit gelu buckets to fg (two-sided) and tanh buckets to ft (odd)."""
    jpath = os.path.join(dstdir, "gelu_and_others.json")
    d = json.load(open(jpath))
    cnt = d["bkt_entry_cnt"]
    bpath = os.path.join(dstdir, "gelu_and_others_bkt.bin")
    bkt = np.fromfile(bpath, dtype=np.float32).reshape(cnt, 8).copy()

    def f32bits(v):
        return int(np.float32(v).view(np.uint32))

    # ---------------- gelu slot (two-sided) -> fg ----------------
    fx = d["func_exp_to_bkt_start_idx"]["gelu"]
    negs = sorted([(int(e), v[0]) for e, v in fx.items()], key=lambda t: t[1])
    poss = sorted([(int(e), v[1]) for e, v in fx.items() if len(v) > 1],
                  key=lambda t: t[1])
    neg_bounds = [s for _, s in negs] + [poss[0][1]]
    pos_bounds = [s for _, s in poss] + [504]
    for side, lst, bounds in (("neg", negs, neg_bounds), ("pos", poss, pos_bounds)):
        for i, (e, start) in enumerate(lst):
            n = bounds[i + 1] - start
            x0s = bkt[start : start + n, 4].astype(np.float64)
            w = abs(x0s[1] - x0s[0]) if n >= 2 else 2.0 ** e
            for j in range(n):
                x0 = float(x0s[j])
                bkt[start + j, 0:4] = _fit_cubic_f(
                    x0 - w / 2, x0 + w / 2, x0, fg).astype(np.float32)
    # gelu special buckets (small-signal, large-signal tails)
    for k, (lo, hi_, x0) in {
        504: (1e-7, 2.0 ** -7, 0.0),
        505: (-(2.0 ** -7), -1e-7, 0.0),
        506: (4.918, hi, (4.918 + hi) / 2),
        507: (-hi, -8.374, -(hi + 8.374) / 2),
    }.items():
        bkt[k, 0:4] = _fit_cubic_f(lo, hi_, x0, fg).astype(np.float32)
        bkt[k, 4] = x0
    for pm in d["profile_meta_data"]:
        if pm["func_name"].startswith("gelu_"):
            pm["fzero_result"] = f32bits(fg(np.zeros(1))[0])
            pm["fpinf_result"] = f32bits(fg(np.full(1, hi))[0])
            pm["fninf_result"] = f32bits(fg(np.full(1, -hi))[0])

    # ---------------- tanh slot (positive side, hw mirrors) -> ft --------
    fxt = d["func_exp_to_bkt_start_idx"]["tanh"]
    tpos = sorted([(int(e), v[0]) for e, v in fxt.items()], key=lambda t: t[1])
    tmeta = None
    for pm in d["profile_meta_data"]:
        if pm["func_name"].startswith("tanh_"):
            tmeta = pm
    assert tmeta is not None
    small_thr = 2.0 ** (tmeta["small_pos_signal_exp_threshold"] - 127)
    large_thr = _thr_from_meta(tmeta["large_pos_signal_exp_threshold"],
                               tmeta["large_pos_signal_mantissa_threshold"])
    if large_thr is None:
        large_thr = 2.0 ** (tpos[-1][0] + 1)
    tb_end = tmeta["pos_small_signal_pwl_control"]      # specials follow
    t_bounds = [s for _, s in tpos] + [tb_end]
    for i, (e, start) in enumerate(tpos):
        n = t_bounds[i + 1] - start
        x0s = bkt[start : start + n, 4].astype(np.float64)
        w = abs(x0s[1] - x0s[0]) if n >= 2 else 2.0 ** e
        for j in range(n):
            x0 = float(x0s[j])
            bkt[start + j, 0:4] = _fit_cubic_f(
                max(x0 - w / 2, 1e-30), x0 + w / 2, x0, ft).astype(np.float32)
    hi_t = max(hi, large_thr * 1.5)
    specials = {
        tmeta["pos_small_signal_pwl_control"]: (1e-9, small_thr, 0.0),
        tmeta["neg_small_signal_pwl_control"]: (-small_thr, -1e-9, 0.0),
        tmeta["pos_large_signal_pwl_control"]:
            (large_thr, hi_t, (large_thr + hi_t) / 2),
        tmeta["neg_large_signal_pwl_control"]:
            (-hi_t, -large_thr, -(large_thr + hi_t) / 2),
    }
    for k, (lo, hi_, x0) in specials.items():
        bkt[k, 0:4] = _fit_cubic_f(lo, hi_, x0, ft).astype(np.float32)
        bkt[k, 4] = x0
    for pm in d["profile_meta_data"]:
        if pm["func_name"].startswith("tanh_"):
            pm["fzero_result"] = f32bits(0.0)
            pm["fpinf_result"] = f32bits(ft(np.full(1, hi_t))[0])
            pm["fninf_result"] = f32bits(ft(np.full(1, -hi_t))[0])

    bkt.tofile(bpath)
    with open(jpath, "w") as f:
        json.dump(d, f)


def _gen_act_tables_v2(fg, ft, hi, key_bytes):
    from neuronxcc.driver.Job import Job
    from neuronxcc.driver.jobs.support.FindActInfo import findActInfoFile

    src_json = findActInfoFile(Job.getPackageDir(), "gen3")
    srcdir = os.path.dirname(src_json)
    tag = hashlib.md5(
        TABLE_VERSION_V2.encode() + key_bytes + repr(float(hi)).encode()
    ).hexdigest()[:10]
    dstdir = os.path.join(tempfile.gettempdir(), f"cnv2_act_{tag}")
    marker = os.path.join(dstdir, "act_info.json")
    if not os.path.isfile(marker):
        tmp = dstdir + ".tmp"
        shutil.rmtree(tmp, ignore_errors=True)
        os.makedirs(tmp)
        for f in os.listdir(srcdir):
            shutil.copyfile(os.path.join(srcdir, f), os.path.join(tmp, f))
        _patch_tables_v2(tmp, fg, ft, hi)
        shutil.rmtree(dstdir, ignore_errors=True)
        try:
            os.rename(tmp, dstdir)
        except OSError:
            if not os.path.isfile(marker):
                raise
    return marker, tag


# ======================================================================
# V2 kernel build
# ======================================================================


def _build_v2(reps, tag, expscale, nbas=NBAS):
    nc = bacc.Bacc(None)
    xs = nc.dram_tensor("xs", [128, 2 * BSV], F32, kind="ExternalInput")
    fpk = nc.dram_tensor(f"fpk8_{tag}", [128, nbas * NRT * 2 * 128], FP8,
                         kind="ExternalInput")
    eb = nc.dram_tensor("eb", [128, NRT], F32, kind="ExternalInput")
    whd = nc.dram_tensor("whd", [128, NRT], F32R, kind="ExternalInput")
    y = nc.dram_tensor("y", [1, BSV], F32, kind="ExternalOutput")

    with tile.TileContext(nc) as tc, ExitStack() as ctx:
        const = ctx.enter_context(tc.tile_pool(name="const", bufs=1))
        gp = ctx.enter_context(tc.tile_pool(name="gp", bufs=2))
        psum = ctx.enter_context(
            tc.tile_pool(name="psum", bufs=1, space=bass.MemorySpace.PSUM)
        )

        xs_t = const.tile([128, 2 * BSV], F32, tag="xs")
        nc.gpsimd.dma_start(xs_t[:], xs[:])
        fpk_t = const.tile([128, nbas * NRT * 2 * 128], FP8, tag="fpk")
        nc.gpsimd.dma_start(fpk_t[:], fpk[:])
        eb_t = const.tile([128, NRT], F32, tag="eb")
        nc.gpsimd.dma_start(eb_t[:], eb[:])
        whd_t = const.tile([128, NRT], F32R, tag="whd")
        nc.gpsimd.dma_start(whd_t[:], whd[:])

        lz = [psum.tile([128, BSV], F32, tag=f"lz{rt}", name=f"lz{rt}")
              for rt in range(NRT)]

        nmm = reps * nbas
        for rep in range(reps):
            xq_t = gp.tile([128, 2 * BSV], FP8, tag="xq")
            nc.vector.tensor_copy(xq_t[:], xs_t[:])
            g1_t = gp.tile([128, 2 * BSV], FP8, tag="g1")
            nc.scalar.activation(g1_t[:], xs_t[:], AF.Gelu)
            gts = [xq_t, g1_t]
            if nbas >= 3:
                g2_t = gp.tile([128, 2 * BSV], FP8, tag="g2")
                nc.scalar.activation(g2_t[:], xs_t[:], AF.Tanh)
                gts.append(g2_t)
            rhs = [t[:].rearrange("p (two b) -> p two b", two=2) for t in gts]
            for s in range(nbas):
                imm = rep * nbas + s
                for rt in range(NRT):
                    lhsT = fpk_t[
                        :, (s * NRT + rt) * 256 : (s * NRT + rt + 1) * 256
                    ].rearrange("p (two m) -> p two m", two=2)
                    nc.tensor.matmul(
                        lz[rt][:, :], lhsT, rhs[s],
                        start=(imm == 0), stop=(imm == nmm - 1),
                        perf_mode=mybir.MatmulPerfMode.DoubleRow,
                    )

        yp = psum.tile([1, BSV], F32, tag="yp")
        for rt in range(NRT):
            z_t = const.tile([128, BSV], F32R, tag=f"z{rt}")
            nc.scalar.activation(z_t[:], lz[rt][:], AF.Exp,
                                 bias=eb_t[:, rt : rt + 1], scale=expscale)
            nc.tensor.matmul(yp[:, :], whd_t[:, rt : rt + 1], z_t[:],
                             start=(rt == 0), stop=(rt == NRT - 1))
        y_sb = const.tile([1, BSV], F32, tag="ysb")
        nc.vector.tensor_copy(y_sb[:], yp[:])
        nc.sync.dma_start(y[:], y_sb[:])

    nc.compile()
    return nc


def _get_nc_v2(reps, tag, expscale, nbas=NBAS):
    key = ("v2", reps, tag, float(expscale), nbas)
    if key not in _cache:
        _cache[key] = _build_v2(reps, tag, expscale, nbas)
    return _cache[key]


# ======================================================================
# V2 host-side basis optimization + fit + packing
# ======================================================================


def _mask_const(inputs):
    mk = np.asarray(inputs["mask_logit"], dtype=np.float64)
    v = mk.reshape(-1)[0]
    return float(v) if np.all(mk == v) else None


def _prep_v2(inputs, nbas=NBAS):
    """Returns (in_maps, tag, hb, expscale) or None if not applicable."""
    mkv = _mask_const(inputs)
    if mkv is None:
        return None
    m = 1.0 / (1.0 + np.exp(-np.float64(mkv)))
    if not (1e-8 < m < 1 - 1e-8):
        return None

    x = np.asarray(inputs["x"], dtype=np.float64)
    sg = np.asarray(inputs["sign_param"], dtype=np.float64)
    th = np.asarray(inputs["th"], dtype=np.float64)
    lk = float(np.asarray(inputs["log_kappa"], dtype=np.float64).reshape(-1)[0])
    hwt = np.asarray(inputs["head_w"], dtype=np.float64).reshape(-1)
    hb = float(np.asarray(inputs["head_b"], dtype=np.float64).reshape(-1)[0])

    kappa = np.exp(lk)
    a = kappa * np.tanh(sg)                          # (R, D)
    if float(np.abs(a * th).max()) > 1e-12:
        # per-element bias b = a*th not supported by the shared-function
        # dictionary; leave to the NPHI-3 fallback (which handles biases)
        return None
    amax = float(np.abs(a).max())
    xmax = float(np.abs(x).max())
    if amax == 0.0:
        return None

    L = max(5.6, 1.05 * xmax)
    NX = 1601
    xg = np.linspace(-L, L, NX)
    hcnt, _ = np.histogram(x.reshape(-1), bins=NX,
                           range=(-L - L/(NX-1)/2, L + L/(NX-1)/2))
    wx = hcnt.astype(np.float64) + hcnt.max() * 1e-3
    wx /= wx.sum()

    rng = np.random.default_rng(0)
    af = a.reshape(-1)
    sel = rng.choice(af.size, min(V2_ASUB, af.size), replace=False)
    Psub = _phi64(af[sel][:, None] * xg[None, :], m)

    def fit_sub(Dm):
        Sdim = Dm.shape[1]
        G = (Dm * wx[:, None]).T @ Dm
        lam = np.zeros(Sdim)
        for s in range(1, Sdim):
            lam[s] = V2_LAM**2 * float((Dm[:, s]**2 * wx).sum())
        K2 = np.linalg.solve(G + np.diag(lam), (Dm * wx[:, None]).T)
        return Psub @ K2.T, G

    def opt_basis(fixed_cols, odd):
        g = _phi64(0.45 * amax * xg, m)
        g = g - (g * wx).sum()
        g /= np.sqrt((g**2 * wx).sum())
        if odd:
            g = 0.5 * (g - g[::-1])
        for _ in range(V2_NITER):
            cols = [fixed_cols[0]] + [
                _q8(c, _scale_exp(np.abs(c).max() + 1e-12))
                for c in fixed_cols[1:] + [g]]
            Dm = np.stack(cols, 1)
            C, _ = fit_sub(Dm)
            T = Psub - C[:, :-1] @ Dm[:, :-1].T
            c = C[:, -1]
            g = (c @ T) / (c @ c)
            if odd:
                g = 0.5 * (g - g[::-1])
            else:
                g = g - (g * wx).sum()
            r = np.sqrt((g**2 * wx).sum())
            g = np.clip(g / r, -V2_CAP, V2_CAP)
            if odd:
                g = 0.5 * (g - g[::-1])
            else:
                g = g - (g * wx).sum()
            g /= np.sqrt((g**2 * wx).sum())
        return g

    one = np.ones_like(xg)
    g1 = opt_basis([one, xg], odd=False)
    g2 = (opt_basis([one, xg, g1], odd=True) if nbas >= 3
          else np.zeros_like(xg))

    # quantized basis columns (exactly the device grid)
    e_x = 0
    e1 = _scale_exp(np.abs(g1).max())
    e2 = _scale_exp(np.abs(g2).max()) if nbas >= 3 else 0
    cols = [one, _q8(xg, e_x), _q8(g1, e1)]
    if nbas >= 3:
        cols.append(_q8(g2, e2))
    Dm = np.stack(cols, 1)
    S = Dm.shape[1]
    G = (Dm * wx[:, None]).T @ Dm
    lam = np.zeros(S)
    for s in range(1, S):
        lam[s] = V2_LAM**2 * float((Dm[:, s]**2 * wx).sum())
    K2 = np.linalg.solve(G + np.diag(lam), (Dm * wx[:, None]).T)

    C = np.empty((af.size, S))
    for i0 in range(0, af.size, 8192):
        P = _phi64(af[i0:i0 + 8192, None] * xg[None, :], m)
        C[i0:i0 + 8192] = P @ K2.T
    cmax = float(np.abs(C[:, 1:]).max())
    ce = _scale_exp(cmax)

    # coordinate-descent rounding to the fp8 grid
    bvec = C @ G
    Cq = C.copy()
    for s in range(1, S):
        Cq[:, s] = _q8(Cq[:, s], ce)
    for _ in range(3):
        for s in range(1, S):
            num = (bvec[:, s] - Cq @ G[:, s] + Cq[:, s] * G[s, s]) / G[s, s]
            Cq[:, s] = _q8(num, ce)
    num0 = bvec[:, 0] - Cq @ G[:, 0] + Cq[:, 0] * G[0, 0]
    Cq[:, 0] = num0 / G[0, 0]

    # residual gate: per-rule logz residual std (grid-weighted)
    res2 = np.empty(af.size)
    for i0 in range(0, af.size, 8192):
        P = _phi64(af[i0:i0 + 8192, None] * xg[None, :], m)
        Rm = P - Cq[i0:i0 + 8192] @ Dm.T
        res2[i0:i0 + 8192] = (Rm * Rm) @ wx
    err_r = np.sqrt(res2.reshape(R, D).sum(axis=1))
    if float(err_r.max()) > V2_GATE:
        return None

    # per-basis product scale folding: F_s = Cq_s * 2^(P - e_s) in fp8
    es = [e_x, e1, e2][:nbas]
    Pexp = min(es[s] + _scale_exp(
        float(np.abs(Cq[:, s + 1]).max()) + 1e-30) for s in range(nbas))
    expscale = float(2.0 ** (-Pexp))

    Cr = Cq.reshape(R, D, S)
    fpk_cols = np.empty((128, nbas * NRT * 2 * 128), dtype=NP8)
    for s in range(nbas):
        F = Cr[:, :, s + 1] * 2.0 ** (Pexp - es[s])       # (R, D)
        t4 = F.reshape(NRT, 128, 2, 128).transpose(3, 0, 2, 1)  # [p,rt,h,j]
        fpk_cols[:, s * NRT * 256 : (s + 1) * NRT * 256] = (
            t4.reshape(128, NRT * 256).astype(np.float32).astype(NP8))

    eb_arr = np.ascontiguousarray(
        Cr[:, :, 0].sum(axis=1).reshape(NRT, 128).T, dtype=np.float32)
    whd_arr = np.ascontiguousarray(
        hwt.reshape(NRT, 128).T, dtype=np.float32)

    # activation tables: gelu := g1 * 2^e1, tanh := g2 * 2^e2
    fg = lambda u: np.interp(u, xg, g1 * 2.0**e1)
    ft = lambda u: np.interp(u, xg, g2 * 2.0**e2)
    key_bytes = (np.float32(g1).tobytes() + np.float32(g2).tobytes()
                 + np.int64([e1, e2, Pexp, nbas]).tobytes())
    json_path, tag = _gen_act_tables_v2(fg, ft, float(L), key_bytes)

    xT = x.T                                          # (D, B)
    in_maps = []
    for c in range(NCORES):
        bsl = slice(c * BSV, (c + 1) * BSV)
        xs_arr = np.concatenate([xT[0:128, bsl], xT[128:256, bsl]], axis=1)
        in_maps.append({
            "xs": np.ascontiguousarray(xs_arr, dtype=np.float32),
            f"fpk8_{tag}": fpk_cols,
            "eb": eb_arr,
            "whd": whd_arr,
        })
    os.environ["BASS_ACT_ROOT_JSON_PATH"] = json_path
    return in_maps, tag, hb, expscale


# ======================================================================
# Dispatch
# ======================================================================


def _run(inputs, reps=1, **spmd_kwargs):
    prep2 = _prep_v2(inputs)
    if prep2 is not None:
        in_maps, tag, hb, expscale = prep2
        nc = _get_nc_v2(reps, tag, expscale)
        res = run_bass_kernel_spmd(nc, in_maps, core_ids=list(range(NCORES)),
                                   **spmd_kwargs)
        y = np.concatenate(
            [np.asarray(r["y"][0], dtype=np.float32) for r in res.results]
        ) + np.float32(hb)
        return y.astype(np.float32), res

    prep = _prep_lr(inputs)
    if prep is not None:
        in_maps, alphas, tag, hb = prep
        nc = _get_nc_lr(reps, alphas, tag)
        res = run_bass_kernel_spmd(nc, in_maps, core_ids=list(range(NCORES)),
                                   **spmd_kwargs)
        y = np.empty(B, dtype=np.float32)
        for gb in range(GB):
            acc = np.zeros(BS, dtype=np.float32)
            for gr in range(GR):
                acc += np.asarray(res.results[gr * GB + gb]["y"][0],
                                  dtype=np.float32)
            y[gb * BS : (gb + 1) * BS] = acc + np.float32(hb)
        return y, res

    os.environ.pop("BASS_ACT_ROOT_JSON_PATH", None)
    nc = _get_nc(reps)
    in_maps = _make_in_maps(inputs)
    res = run_bass_kernel_spmd(nc, in_maps, core_ids=list(range(NCORES)),
                               **spmd_kwargs)
    hb = np.asarray(inputs["head_b"], dtype=np.float32).reshape(-1)[0]
    y = np.sum([r["y"][0] for r in res.results], axis=0, dtype=np.float32) + hb
    return y.astype(np.float32), res


def kernel(**inputs) -> np.ndarray:
    y, _ = _run(inputs)
    return y



# revision 10
# speedup vs baseline: 1.8773x; 1.8773x over previous
"""CornerNet Trainium2 kernel — fp8 DoubleRow + designed act-table bases.

Math (reference):
  t     = kappa * tanh(sign_param) * (x - th)        # (B, R, D)
  s     = sigmoid(t); m = sigmoid(mask_logit)
  gated = 1 - m*(1-s)
  z     = prod_d gated                               # (B, R)
  y     = z @ head_w.T + head_b                      # (B,)

V2 FAST PATH (uniform mask_logit, th == 0 — the actual model):
  log z[b,r] = sum_d phi(a[r,d]*x[b,d]),  phi(t) = ln(1-m+m*sigmoid(t)).
  Per-element weighted-LSQ fit onto the dictionary { 1, x, v1(x) }
  (optionally + v2(x), odd, at nbas=3), where v1/v2 are FREE functions
  optimized on the host by greedy alternating least squares over the
  empirical (a, x) distribution, amplitude-capped so fp8 noise is not
  tail-amplified, then baked into the `gelu` (+`tanh`) activation-table
  slots (NEFF embeds them via BASS_ACT_ROOT_JSON_PATH; the tanh slot is
  hardware-mirrored, hence v2 odd).  All matmul operands are fp8e4m3 with
  power-of-2 scales folded into tables/coefficients, so TensorE runs
  MatmulPerfMode.DoubleRow (2 fp8 contraction rows/cycle, K=256 per
  instruction).  Coefficients are fit quantization-aware: basis columns
  pre-quantized to the device fp8 grid, ridge = fp8 rounding noise
  variance, then coordinate-descent rounding onto the fp8 lattice with an
  exactly-kept constant term (folded into the Exp bias).

  Sharding: data-parallel over batch, 8 x 256; every core holds all 512
  rules, so per-core outputs concatenate with no cross-core reduction.
  Per core per rep: 1 ScalarE table pass + 1 DVE fp8 cast + nbas*4
  DoubleRow matmuls into 4 PSUM tiles; epilogue (once): per-rule-tile Exp
  (bias = constant term, scale = 2^-P) and a 4-tile f32r head matmul.
  HW ~0.6us/rep at nbas=2 (y rel err ~1.2e-2), ~1.1us at nbas=3
  (~0.9e-2), vs 2.9us for the f32r NPHI=3 fallback below (~1.4e-3).

V1 FALLBACK (kept for poor-fit inputs — nonzero th etc.):
  log z[b,r] = sum_d phi(a[r,d]*x[b,d] - b[r,d]),
  phi(t) = ln(1 - m + m*sigmoid(t)),  a = kappa*tanh(sign_param), b = a*th.

  phi(a*x - b) is approximated per (r,d) element by a weighted least-squares
  fit onto a small dictionary of functions of x alone:
      { 1, x, phi(alpha_1 x), ..., phi(alpha_NPHI x) }
  so  log z = C_const (summed over d, folded into the final Exp bias)
            + sum_{k in basis} F_k @ G_k(x),
  i.e. 1+NPHI matmuls on TensorE instead of B*R*D scalar-engine work (y rel
  err ~1e-3 at NPHI=3, fit-residual-checked on the host per call).  The
  dictionary coefficients are computed exactly per element on the host
  (adapting to the actual kappa/th/sign_param/x distribution), and
  phi(alpha x) is ONE ScalarE pass per alpha via the gelu activation-table
  slot re-fitted to phi_m (the NEFF embeds the tables;
  BASS_ACT_ROOT_JSON_PATH selects them).  The scales were optimized by
  multi-start coordinate descent on the family fit residual, which also
  keeps the coefficients small (max|C| ~4) so table-approximation error is
  not amplified.  If the inputs are outside what the dictionary can
  represent the kernel falls back to the exact per-rule path below.

  Sharding: 2 rule-groups x 4 batch-groups over 8 cores.  Per core: 256
  rules (2 PSUM tiles of 128) x 512 batch, D=256 on partitions in 2 halves.
  Per rep per core: NPHI ScalarE activation passes + 4*(1+NPHI) f32r
  matmuls (512 rows each) + final Exp (bias = constant term) and a tiny
  head matmul.  At NPHI=3: TensorE ~3.4us and ScalarE ~3.1us per rep —
  both engines near their streaming floors.

FALLBACK (non-uniform mask_logit or poor fit): per-rule sigmoid/ln path,
tensor-parallel over rules (8x64), ScalarE-bound (~343us).
"""

import numpy as np
import math
from contextlib import ExitStack

import ml_dtypes

import concourse.bass as bass
import concourse.bacc as bacc
import concourse.mybir as mybir
import concourse.tile as tile
from concourse.bass_utils import run_bass_kernel_spmd
from bass_rust import add_dep_helper

B, D, R = 2048, 256, 512
NCORES = 8
F32 = mybir.dt.float32
F32R = mybir.dt.float32r
FP8 = mybir.dt.float8e4
NP8 = ml_dtypes.float8_e4m3
AF = mybir.ActivationFunctionType
OP = mybir.AluOpType

# ---- low-rank fast path geometry ----
GR, GB = 2, 4            # rule groups x batch groups
RS = R // GR             # 256 rules per core (2 tiles of 128)
BS = B // GB             # 512 batch per core
POWERS = ()              # extra monomial basis (computed on DVE)
NPHI = 3
NB = 1 + len(POWERS) + NPHI   # matmul basis: x, powers, phi(alpha_k x)
# multi-start coordinate-polished scale ratios (relative to max|a|); the
# polish also shrinks the fit coefficients (max|C| ~4 vs ~180), which keeps
# the activation-table approximation error of phi from being amplified
RATIOS = (0.3177, -0.3388, 0.6539)

_cache = {}


# ======================================================================
# Activation tables: refit the `gelu` spline buckets to phi_m.
# ======================================================================

import hashlib
import json
import os
import shutil
import tempfile

TABLE_VERSION = "v2"


def _phi64(u, m):
    c = 1.0 - m
    u = np.asarray(u, np.float64)
    return np.logaddexp(np.log(c), u) - np.logaddexp(0.0, u)


def _fit_cubic(lo, hi, x0, m):
    u = np.linspace(lo, hi, 129)
    y = _phi64(u, m)
    A = np.vander(u - x0, 4, increasing=True)
    coef, *_ = np.linalg.lstsq(A, y, rcond=None)
    return coef


def _patch_gelu_tables(dstdir, m, hi):
    jpath = os.path.join(dstdir, "gelu_and_others.json")
    d = json.load(open(jpath))
    cnt = d["bkt_entry_cnt"]
    bpath = os.path.join(dstdir, "gelu_and_others_bkt.bin")
    bkt = np.fromfile(bpath, dtype=np.float32).reshape(cnt, 8).copy()

    fx = d["func_exp_to_bkt_start_idx"]["gelu"]
    negs = sorted([(int(e), v[0]) for e, v in fx.items()], key=lambda t: t[1])
    poss = sorted([(int(e), v[1]) for e, v in fx.items() if len(v) > 1],
                  key=lambda t: t[1])
    neg_bounds = [s for _, s in negs] + [poss[0][1]]
    pos_bounds = [s for _, s in poss] + [504]

    for side, lst, bounds in (("neg", negs, neg_bounds), ("pos", poss, pos_bounds)):
        for i, (e, start) in enumerate(lst):
            n = bounds[i + 1] - start
            x0s = bkt[start : start + n, 4].astype(np.float64)
            if n >= 2:
                w = abs(x0s[1] - x0s[0])
            else:
                w = 2.0 ** e
            for j in range(n):
                x0 = float(x0s[j])
                lo, hi_ = x0 - w / 2, x0 + w / 2
                bkt[start + j, 0:4] = _fit_cubic(lo, hi_, x0, m).astype(np.float32)
    # special buckets: small-signal (|u|<2^-7) and large-signal tails.
    # gelu profile thresholds: pos-large 4.918, neg-large -8.374.  The tails
    # must cover max|alpha*x| (phi is ~flat there so one cubic is plenty).
    for k, (lo, hi_, x0) in {
        504: (1e-7, 2.0 ** -7, 0.0),
        505: (-(2.0 ** -7), -1e-7, 0.0),
        506: (4.918, hi, (4.918 + hi) / 2),
        507: (-hi, -8.374, -(hi + 8.374) / 2),
    }.items():
        bkt[k, 0:4] = _fit_cubic(lo, hi_, x0, m).astype(np.float32)
        bkt[k, 4] = x0
    bkt.tofile(bpath)

    def f32bits(v):
        return int(np.float32(v).view(np.uint32))

    for pm in d["profile_meta_data"]:
        if pm["func_name"].startswith("gelu_"):
            pm["fzero_result"] = f32bits(_phi64(0.0, m))
            pm["fpinf_result"] = 0
            pm["fninf_result"] = f32bits(np.log(1.0 - m))
    with open(jpath, "w") as f:
        json.dump(d, f)


def _gen_act_tables(m, hi):
    """Build a patched act-table dir (gelu := phi_m); returns (json_path, tag)."""
    from neuronxcc.driver.Job import Job
    from neuronxcc.driver.jobs.support.FindActInfo import findActInfoFile

    src_json = findActInfoFile(Job.getPackageDir(), "gen3")
    srcdir = os.path.dirname(src_json)
    tag = hashlib.md5(
        (TABLE_VERSION + repr(float(np.float64(m))) + repr(float(hi))).encode()
    ).hexdigest()[:10]
    dstdir = os.path.join(tempfile.gettempdir(), f"cn_act_{tag}")
    marker = os.path.join(dstdir, "act_info.json")
    if not os.path.isfile(marker):
        tmp = dstdir + ".tmp"
        shutil.rmtree(tmp, ignore_errors=True)
        os.makedirs(tmp)
        for f in os.listdir(srcdir):
            shutil.copyfile(os.path.join(srcdir, f), os.path.join(tmp, f))
        _patch_gelu_tables(tmp, m, hi)
        shutil.rmtree(dstdir, ignore_errors=True)
        try:
            os.rename(tmp, dstdir)
        except OSError:
            if not os.path.isfile(marker):
                raise
    return marker, tag


# ======================================================================
# Low-rank kernel build
# ======================================================================


def _build_lr(reps, alphas, tag):
    nc = bacc.Bacc(None)
    xs = nc.dram_tensor("xs", [128, 2 * BS], F32R, kind="ExternalInput")
    fpk = nc.dram_tensor(f"fpk_{tag}", [128, NB * 4 * 128], F32R,
                         kind="ExternalInput")
    eb = nc.dram_tensor("eb", [128, GR], F32, kind="ExternalInput")
    whd = nc.dram_tensor("whd", [128, GR], F32R, kind="ExternalInput")
    y = nc.dram_tensor("y", [1, BS], F32, kind="ExternalOutput")

    with tile.TileContext(nc) as tc, ExitStack() as ctx:
        const = ctx.enter_context(tc.tile_pool(name="const", bufs=1))
        gp = ctx.enter_context(tc.tile_pool(name="gp", bufs=4))
        pp = ctx.enter_context(tc.tile_pool(name="pp", bufs=2))
        psum = ctx.enter_context(
            tc.tile_pool(name="psum", bufs=1, space=bass.MemorySpace.PSUM)
        )

        xs_t = const.tile([128, 2 * BS], F32R, tag="xs")
        nc.gpsimd.dma_start(xs_t[:], xs[:])
        fpk_t = const.tile([128, NB * 4 * 128], F32R, tag="fpk")
        for q in range(4):
            w = NB * 128
            nc.gpsimd.dma_start(fpk_t[:, q * w : (q + 1) * w],
                                fpk[:, q * w : (q + 1) * w])
        eb_t = const.tile([128, GR], F32, tag="eb")
        nc.gpsimd.dma_start(eb_t[:], eb[:])
        whd_t = const.tile([128, GR], F32R, tag="whd")
        nc.gpsimd.dma_start(whd_t[:], whd[:])

        lz0 = psum.tile([128, BS], F32, tag="lz0")
        lz1 = psum.tile([128, BS], F32, tag="lz1")
        lz = [lz0, lz1]

        npow = len(POWERS)
        nmm = reps * NB * 2          # matmuls per lz tile (halves x basis x reps)
        imm = 0
        for rep in range(reps):
            rhs_tiles = {0: xs_t}
            prev = xs_t
            for pi in range(npow):
                pw_t = pp.tile([128, 2 * BS], F32R, tag=f"pw{pi}")
                nc.vector.tensor_mul(pw_t[:], prev[:], xs_t[:])
                rhs_tiles[1 + pi] = pw_t
                prev = pw_t
            for k in range(NPHI):
                g = gp.tile([128, 2 * BS], F32R, tag="g")
                nc.scalar.activation(g[:], xs_t[:], AF.Gelu, scale=alphas[k])
                rhs_tiles[1 + npow + k] = g
            for bi in range(NB):
                rhs = rhs_tiles[bi]
                for h in range(2):
                    for rt in range(GR):
                        nc.tensor.matmul(
                            lz[rt][:, :],
                            fpk_t[:, (bi * 4 + h * 2 + rt) * 128 :
                                  (bi * 4 + h * 2 + rt + 1) * 128],
                            rhs[:, h * BS : (h + 1) * BS],
                            start=(imm // 2 == 0),
                            stop=(imm // 2 == nmm - 1),
                        )
                        imm += 1

        yp = psum.tile([1, BS], F32, tag="yp")
        for rt in range(GR):
            z_t = const.tile([128, BS], F32R, tag=f"z{rt}")
            nc.scalar.activation(z_t[:], lz[rt][:], AF.Exp,
                                 bias=eb_t[:, rt : rt + 1])
            nc.tensor.matmul(yp[:, :], whd_t[:, rt : rt + 1], z_t[:],
                             start=(rt == 0), stop=(rt == GR - 1))
        y_sb = const.tile([1, BS], F32, tag="ysb")
        nc.vector.tensor_copy(y_sb[:], yp[:])
        nc.sync.dma_start(y[:], y_sb[:])

    nc.compile()
    return nc


def _get_nc_lr(reps, alphas, tag):
    key = ("lr", reps, alphas, tag)
    if key not in _cache:
        _cache[key] = _build_lr(reps, alphas, tag)
    return _cache[key]


# ======================================================================
# Host-side fit + packing
# ======================================================================


def _mask_const(inputs):
    mk = np.asarray(inputs["mask_logit"], dtype=np.float64)
    v = mk.reshape(-1)[0]
    return float(v) if np.all(mk == v) else None


def _prep_lr(inputs):
    """Fit dictionary coefficients; returns (in_maps, alphas, tag, hb) or
    None if the fast path doesn't apply."""
    mkv = _mask_const(inputs)
    if mkv is None:
        return None
    m = 1.0 / (1.0 + np.exp(-np.float64(mkv)))
    if not (1e-8 < m < 1 - 1e-8):
        return None

    x = np.asarray(inputs["x"], dtype=np.float64)
    sg = np.asarray(inputs["sign_param"], dtype=np.float64)
    th = np.asarray(inputs["th"], dtype=np.float64)
    lk = float(np.asarray(inputs["log_kappa"], dtype=np.float64).reshape(-1)[0])
    hwt = np.asarray(inputs["head_w"], dtype=np.float64).reshape(-1)
    hb = float(np.asarray(inputs["head_b"], dtype=np.float64).reshape(-1)[0])

    kappa = np.exp(lk)
    a = kappa * np.tanh(sg)                 # (R, D)
    bb = a * th                             # (R, D) per-element bias
    amax = float(np.abs(a).max())
    xmax = float(np.abs(x).max())
    if amax == 0.0:
        return None
    alphas = tuple(round(r * amax, 4) for r in RATIOS)
    hi = float(max(12.0, np.ceil(1.10 * max(abs(al) for al in alphas) * xmax)))

    # ---- fit grid: empirical x-weights ----
    L = max(5.6, 1.05 * xmax)
    NX = 1601
    xg = np.linspace(-L, L, NX)
    hcnt, _ = np.histogram(x.reshape(-1), bins=NX,
                           range=(-L - L / (NX - 1) / 2, L + L / (NX - 1) / 2))
    wx = hcnt.astype(np.float64) + hcnt.max() * 1e-3
    wx /= wx.sum()

    cols = [np.ones_like(xg), xg] + [xg**p for p in POWERS]
    for al in alphas:
        cols.append(_phi64(al * xg, m))
    Dm = np.stack(cols, 1)                  # (NX, S)  S = 1 + NB
    S = Dm.shape[1]
    G = (Dm * wx[:, None]).T @ Dm
    G += 1e-13 * np.trace(G) * np.eye(S) / S
    K = np.linalg.solve(G, (Dm * wx[:, None]).T)   # (S, NX)

    af = a.reshape(-1)
    bf = bb.reshape(-1)
    C = np.empty((af.size, S))
    res2 = np.empty(af.size)
    for i0 in range(0, af.size, 8192):
        asl = af[i0:i0 + 8192]
        bsl = bf[i0:i0 + 8192]
        P = _phi64(asl[:, None] * xg[None, :] - bsl[:, None], m)
        Cc = P @ K.T
        C[i0:i0 + 8192] = Cc
        Rm = P - Cc @ Dm.T
        res2[i0:i0 + 8192] = (Rm * Rm) @ wx
    # per-rule predicted logz error std: sqrt(sum_d res2)
    err_r = np.sqrt(res2.reshape(R, D).sum(axis=1))
    if err_r.max() > 2.5e-3:
        return None

    C = C.reshape(R, D, S)

    # ---- pack per-core operands ----
    xT = x.T                                 # (D, B)
    in_maps = []
    for c in range(NCORES):
        gr, gb = c // GB, c % GB
        bsl = slice(gb * BS, (gb + 1) * BS)
        xs_arr = np.concatenate([xT[0:128, bsl], xT[128:256, bsl]], axis=1)
        sub = C[gr * RS : (gr + 1) * RS]     # (RS, D, S)
        # [rt, j, h, p, s] -> [p, s-1(bi), h, rt, j]
        t5 = sub.reshape(GR, 128, 2, 128, S).transpose(3, 4, 2, 0, 1)
        fpk_arr = np.ascontiguousarray(
            t5[:, 1:, :, :, :].reshape(128, NB * 4 * 128), dtype=np.float32)
        eb_arr = np.ascontiguousarray(
            sub[:, :, 0].sum(axis=1).reshape(GR, 128).T, dtype=np.float32)
        whd_arr = np.ascontiguousarray(
            hwt[gr * RS : (gr + 1) * RS].reshape(GR, 128).T, dtype=np.float32)
        m_ = {
            "xs": np.ascontiguousarray(xs_arr, dtype=np.float32),
            "eb": eb_arr,
            "whd": whd_arr,
            "__fpk": fpk_arr,
        }
        in_maps.append(m_)
    json_path, tag = _gen_act_tables(m, hi)
    for m_ in in_maps:
        m_[f"fpk_{tag}"] = m_.pop("__fpk")
    os.environ["BASS_ACT_ROOT_JSON_PATH"] = json_path
    return in_maps, alphas, tag, hb


# ======================================================================
# Generic fallback (per-rule sigmoid/ln path) — unchanged baseline
# ======================================================================

RC = R // NCORES
KBLK = 8
CH = 512


def _build(reps=1):
    nc = bacc.Bacc(None)
    xT = nc.dram_tensor("xT", [D, B], F32, kind="ExternalInput")
    thT = nc.dram_tensor("thT", [D, RC], F32, kind="ExternalInput")
    sgT = nc.dram_tensor("sgT", [D, RC], F32, kind="ExternalInput")
    mkT = nc.dram_tensor("mkT", [D, RC], F32, kind="ExternalInput")
    lkb = nc.dram_tensor("lkb", [128, 1], F32, kind="ExternalInput")
    wcol = nc.dram_tensor("wcol", [RC, 1], F32, kind="ExternalInput")
    selp = nc.dram_tensor("selp", [128, 2 * RC], F32R, kind="ExternalInput")
    y = nc.dram_tensor("y", [1, B], F32, kind="ExternalOutput")

    with tile.TileContext(nc) as tc, ExitStack() as ctx:
        const = ctx.enter_context(tc.tile_pool(name="const", bufs=1))
        sp = ctx.enter_context(tc.tile_pool(name="sp", bufs=2))
        gp_ = ctx.enter_context(tc.tile_pool(name="gp_", bufs=2))
        gpp = ctx.enter_context(tc.tile_pool(name="gpp", bufs=KBLK + 1))
        lp = ctx.enter_context(tc.tile_pool(name="lp", bufs=2))
        psum = ctx.enter_context(
            tc.tile_pool(name="psum", bufs=1, space=bass.MemorySpace.PSUM)
        )

        xt = []
        for h in range(2):
            t_ = const.tile([128, B], F32, tag=f"xt{h}")
            nc.gpsimd.dma_start(t_[:], xT[h * 128 : (h + 1) * 128, :])
            xt.append(t_)

        tht, sgt, mkt = [], [], []
        for name, dram, lst in (("th", thT, tht), ("sg", sgT, sgt), ("mk", mkT, mkt)):
            for h in range(2):
                t_ = const.tile([128, RC], F32, tag=f"{name}{h}")
                nc.gpsimd.dma_start(t_[:], dram[h * 128 : (h + 1) * 128, :])
                lst.append(t_)

        lkt = const.tile([128, 1], F32, tag="lkt")
        nc.gpsimd.dma_start(lkt[:], lkb[:])
        selpt = const.tile([128, 2 * RC], F32R, tag="selpt")
        nc.gpsimd.dma_start(selpt[:], selp[:])
        wct = const.tile([RC, 1], F32, tag="wct")
        nc.gpsimd.dma_start(wct[:], wcol[:])

        kap = const.tile([128, 1], F32, tag="kap")
        nc.scalar.activation(kap[:], lkt[:], AF.Exp)
        nkap = const.tile([128, 1], F32, tag="nkap")
        nc.vector.tensor_scalar(nkap[:], kap[:], -1.0, None, OP.mult)

        aa, nb2, mm_, cc_ = [], [], [], []
        for h in range(2):
            tnh = const.tile([128, RC], F32, tag=f"tnh{h}")
            nc.scalar.activation(tnh[:], sgt[h][:], AF.Tanh)
            a_h = const.tile([128, RC], F32, tag=f"a{h}")
            nc.vector.tensor_scalar(a_h[:], tnh[:], kap[:], None, OP.mult)
            na_h = const.tile([128, RC], F32, tag=f"na{h}")
            nc.vector.tensor_scalar(na_h[:], tnh[:], nkap[:], None, OP.mult)
            nb2_h = const.tile([128, RC], F32, tag=f"nb2{h}")
            nc.vector.tensor_mul(nb2_h[:], na_h[:], tht[h][:])
            aa.append(a_h)
            nb2.append(nb2_h)
            m_h = const.tile([128, RC], F32, tag=f"m{h}")
            nc.scalar.activation(m_h[:], mkt[h][:], AF.Sigmoid)
            c_h = const.tile([128, RC], F32, tag=f"c{h}")
            nc.scalar.activation(c_h[:], mkt[h][:], AF.Sigmoid, scale=-1.0)
            mm_.append(m_h)
            cc_.append(c_h)

        lz = psum.tile([RC, B], F32, tag="lz")
        last_ln = None
        for rep in range(reps):
            for blk in range(RC // KBLK):
                gps = []
                sig_insts = []
                for k in range(KBLK):
                    r = blk * KBLK + k
                    s = sp.tile([128, 2 * B], F32, tag="s")
                    for h in range(2):
                        si = nc.scalar.activation(
                            s[:, h * B : (h + 1) * B],
                            xt[h][:],
                            AF.Sigmoid,
                            bias=nb2[h][:, r : r + 1],
                            scale=aa[h][:, r : r + 1],
                        )
                        if last_ln is not None:
                            add_dep_helper(si.ins, last_ln.ins, False,
                                           "act-table phase blocking")
                        sig_insts.append(si)
                    g = gp_.tile([128, 2 * B], F32, tag="g")
                    for h in range(2):
                        nc.vector.tensor_scalar(
                            g[:, h * B : (h + 1) * B],
                            s[:, h * B : (h + 1) * B],
                            mm_[h][:, r : r + 1],
                            cc_[h][:, r : r + 1],
                            OP.mult,
                            OP.add,
                        )
                    gpt = gpp.tile([128, B], F32, tag="gpt")
                    nc.vector.tensor_mul(gpt[:], g[:, 0:B], g[:, B : 2 * B])
                    gps.append(gpt)
                for k in range(KBLK):
                    r = blk * KBLK + k
                    L = lp.tile([128, B], F32R, tag="L")
                    ln_i = nc.scalar.activation(L[:], gps[k][:], AF.Ln)
                    add_dep_helper(ln_i.ins, sig_insts[-1].ins, False,
                                   "act-table phase blocking")
                    last_ln = ln_i
                    lhsp = selpt[:, RC - r : 2 * RC - r]
                    for c in range(B // CH):
                        nc.tensor.matmul(
                            lz[:, c * CH : (c + 1) * CH],
                            lhsp,
                            L[:, c * CH : (c + 1) * CH],
                            start=(r == 0 and rep == 0),
                            stop=(r == RC - 1 and rep == reps - 1),
                        )

        z_sb = const.tile([RC, B], F32, tag="z")
        nc.scalar.activation(z_sb[:], lz[:], AF.Exp)
        yp = psum.tile([1, B], F32, tag="yp")
        for c in range(B // CH):
            nc.tensor.matmul(
                yp[:, c * CH : (c + 1) * CH],
                wct[:],
                z_sb[:, c * CH : (c + 1) * CH],
                start=True,
                stop=True,
            )
        y_sb = const.tile([1, B], F32, tag="ysb")
        nc.vector.tensor_copy(y_sb[:], yp[:])
        nc.sync.dma_start(y[:], y_sb[:])

    nc.compile()
    return nc


def _get_nc(reps=1):
    key = ("nc", reps)
    if key not in _cache:
        _cache[key] = _build(reps)
    return _cache[key]


def _make_in_maps(inputs):
    x = np.ascontiguousarray(inputs["x"], dtype=np.float32)
    th = np.asarray(inputs["th"], dtype=np.float32)
    sg = np.asarray(inputs["sign_param"], dtype=np.float32)
    mk = np.asarray(inputs["mask_logit"], dtype=np.float32)
    lk = float(np.asarray(inputs["log_kappa"], dtype=np.float32).reshape(-1)[0])
    hw = np.asarray(inputs["head_w"], dtype=np.float32)

    xT = np.ascontiguousarray(x.T)
    lkb = np.full((128, 1), lk, dtype=np.float32)
    selp = np.zeros((128, 2 * RC), dtype=np.float32)
    selp[:, RC] = 1.0

    in_maps = []
    for c in range(NCORES):
        sl = slice(c * RC, (c + 1) * RC)
        in_maps.append(
            {
                "xT": xT,
                "thT": np.ascontiguousarray(th[sl].T),
                "sgT": np.ascontiguousarray(sg[sl].T),
                "mkT": np.ascontiguousarray(mk[sl].T),
                "lkb": lkb,
                "wcol": np.ascontiguousarray(hw.reshape(-1)[sl].reshape(RC, 1)),
                "selp": selp,
            }
        )
    return in_maps


# ======================================================================
# V2 path: fp8 DoubleRow + designed act-table bases (see module doc)
# ======================================================================

# ---- v2 geometry: data-parallel over batch ----
BSV = B // NCORES        # 256 batch per core
NBAS = 2                 # matmul bases: x, v1 (v2 available at nbas=3)
NRT = R // 128           # 4 rule tiles

# ---- v2 fit hyperparameters ----
V2_CAP = 4.0             # basis amplitude cap (fp8 noise tail control)
V2_LAM = 0.03            # ridge = fp8 C-noise rms (relative)
V2_NITER = 16            # alternating iterations per basis
V2_ASUB = 12288          # a-subsample for basis optimization
V2_GATE = 1.35e-2        # max per-rule logz residual std to accept

import hashlib
import json
import os
import shutil
import tempfile

TABLE_VERSION_V2 = "v2t1"


def _phi64(u, m):
    c = 1.0 - m
    u = np.asarray(u, np.float64)
    return np.logaddexp(np.log(c), u) - np.logaddexp(0.0, u)


def _q8(v, e):
    """fp8e4m3 (trn float8e4) quantize with scale 2**e, dequantized float."""
    return (np.asarray(v * (2.0**e), np.float32).astype(NP8).astype(np.float64)
            ) * 2.0 ** (-e)


def _scale_exp(maxabs):
    if maxabs <= 0:
        return 0
    return int(math.floor(math.log2(240.0 / maxabs)))


# ======================================================================
# Activation tables: gelu := fg, tanh := ft (ft odd; hw mirrors sign)
# ======================================================================


def _fit_cubic_f(lo, hi, x0, f):
    u = np.linspace(lo, hi, 129)
    y = f(u)
    A = np.vander(u - x0, 4, increasing=True)
    coef, *_ = np.linalg.lstsq(A, y, rcond=None)
    return coef


def _thr_from_meta(exp_thr, mant_thr):
    if exp_thr == 0:
        return None
    return 2.0 ** (exp_thr - 127) * (1.0 + mant_thr / 2.0**23)


def _patch_tables_v2(dstdir, fg, ft, hi):
    """Refit gelu buckets to fg (two-sided) and tanh buckets to ft (odd)."""
    jpath = os.path.join(dstdir, "gelu_and_others.json")
    d = json.load(open(jpath))
    cnt = d["bkt_entry_cnt"]
    bpath = os.path.join(dstdir, "gelu_and_others_bkt.bin")
    bkt = np.fromfile(bpath, dtype=np.float32).reshape(cnt, 8).copy()

    def f32bits(v):
        return int(np.float32(v).view(np.uint32))

    # ---------------- gelu slot (two-sided) -> fg ----------------
    fx = d["func_exp_to_bkt_start_idx"]["gelu"]
    negs = sorted([(int(e), v[0]) for e, v in fx.items()], key=lambda t: t[1])
    poss = sorted([(int(e), v[1]) for e, v in fx.items() if len(v) > 1],
                  key=lambda t: t[1])
    neg_bounds = [s for _, s in negs] + [poss[0][1]]
    pos_bounds = [s for _, s in poss] + [504]
    for side, lst, bounds in (("neg", negs, neg_bounds), ("pos", poss, pos_bounds)):
        for i, (e, start) in enumerate(lst):
            n = bounds[i + 1] - start
            x0s = bkt[start : start + n, 4].astype(np.float64)
            w = abs(x0s[1] - x0s[0]) if n >= 2 else 2.0 ** e
            for j in range(n):
                x0 = float(x0s[j])
                bkt[start + j, 0:4] = _fit_cubic_f(
                    x0 - w / 2, x0 + w / 2, x0, fg).astype(np.float32)
    # gelu special buckets (small-signal, large-signal tails)
    for k, (lo, hi_, x0) in {
        504: (1e-7, 2.0 ** -7, 0.0),
        505: (-(2.0 ** -7), -1e-7, 0.0),
        506: (4.918, hi, (4.918 + hi) / 2),
        507: (-hi, -8.374, -(hi + 8.374) / 2),
    }.items():
        bkt[k, 0:4] = _fit_cubic_f(lo, hi_, x0, fg).astype(np.float32)
        bkt[k, 4] = x0
    for pm in d["profile_meta_data"]:
        if pm["func_name"].startswith("gelu_"):
            pm["fzero_result"] = f32bits(fg(np.zeros(1))[0])
            pm["fpinf_result"] = f32bits(fg(np.full(1, hi))[0])
            pm["fninf_result"] = f32bits(fg(np.full(1, -hi))[0])

    # ---------------- tanh slot (positive side, hw mirrors) -> ft --------
    fxt = d["func_exp_to_bkt_start_idx"]["tanh"]
    tpos = sorted([(int(e), v[0]) for e, v in fxt.items()], key=lambda t: t[1])
    tmeta = None
    for pm in d["profile_meta_data"]:
        if pm["func_name"].startswith("tanh_"):
            tmeta = pm
    assert tmeta is not None
    small_thr = 2.0 ** (tmeta["small_pos_signal_exp_threshold"] - 127)
    large_thr = _thr_from_meta(tmeta["large_pos_signal_exp_threshold"],
                               tmeta["large_pos_signal_mantissa_threshold"])
    if large_thr is None:
        large_thr = 2.0 ** (tpos[-1][0] + 1)
    tb_end = tmeta["pos_small_signal_pwl_control"]      # specials follow
    t_bounds = [s for _, s in tpos] + [tb_end]
    for i, (e, start) in enumerate(tpos):
        n = t_bounds[i + 1] - start
        x0s = bkt[start : start + n, 4].astype(np.float64)
        w = abs(x0s[1] - x0s[0]) if n >= 2 else 2.0 ** e
        for j in range(n):
            x0 = float(x0s[j])
            bkt[start + j, 0:4] = _fit_cubic_f(
                max(x0 - w / 2, 1e-30), x0 + w / 2, x0, ft).astype(np.float32)
    hi_t = max(hi, large_thr * 1.5)
    specials = {
        tmeta["pos_small_signal_pwl_control"]: (1e-9, small_thr, 0.0),
        tmeta["neg_small_signal_pwl_control"]: (-small_thr, -1e-9, 0.0),
        tmeta["pos_large_signal_pwl_control"]:
            (large_thr, hi_t, (large_thr + hi_t) / 2),
        tmeta["neg_large_signal_pwl_control"]:
            (-hi_t, -large_thr, -(large_thr + hi_t) / 2),
    }
    for k, (lo, hi_, x0) in specials.items():
        bkt[k, 0:4] = _fit_cubic_f(lo, hi_, x0, ft).astype(np.float32)
        bkt[k, 4] = x0
    for pm in d["profile_meta_data"]:
        if pm["func_name"].startswith("tanh_"):
            pm["fzero_result"] = f32bits(0.0)
            pm["fpinf_result"] = f32bits(ft(np.full(1, hi_t))[0])
            pm["fninf_result"] = f32bits(ft(np.full(1, -hi_t))[0])

    bkt.tofile(bpath)
    with open(jpath, "w") as f:
        json.dump(d, f)


def _gen_act_tables_v2(fg, ft, hi, key_bytes):
    from neuronxcc.driver.Job import Job
    from neuronxcc.driver.jobs.support.FindActInfo import findActInfoFile

    src_json = findActInfoFile(Job.getPackageDir(), "gen3")
    srcdir = os.path.dirname(src_json)
    tag = hashlib.md5(
        TABLE_VERSION_V2.encode() + key_bytes + repr(float(hi)).encode()
    ).hexdigest()[:10]
    dstdir = os.path.join(tempfile.gettempdir(), f"cnv2_act_{tag}")
    marker = os.path.join(dstdir, "act_info.json")
    if not os.path.isfile(marker):
        tmp = dstdir + ".tmp"
        shutil.rmtree(tmp, ignore_errors=True)
        os.makedirs(tmp)
        for f in os.listdir(srcdir):
            shutil.copyfile(os.path.join(srcdir, f), os.path.join(tmp, f))
        _patch_tables_v2(tmp, fg, ft, hi)
        shutil.rmtree(dstdir, ignore_errors=True)
        try:
            os.rename(tmp, dstdir)
        except OSError:
            if not os.path.isfile(marker):
                raise
    return marker, tag


# ======================================================================
# V2 kernel build
# ======================================================================


def _build_v2(reps, tag, expscale, nbas=NBAS):
    nc = bacc.Bacc(None)
    xs = nc.dram_tensor("xs", [128, 2 * BSV], F32, kind="ExternalInput")
    fpk = nc.dram_tensor(f"fpk8_{tag}", [128, nbas * NRT * 2 * 128], FP8,
                         kind="ExternalInput")
    eb = nc.dram_tensor("eb", [128, NRT], F32, kind="ExternalInput")
    whd = nc.dram_tensor("whd", [128, NRT], F32R, kind="ExternalInput")
    y = nc.dram_tensor("y", [1, BSV], F32, kind="ExternalOutput")

    with tile.TileContext(nc) as tc, ExitStack() as ctx:
        const = ctx.enter_context(tc.tile_pool(name="const", bufs=1))
        gp = ctx.enter_context(tc.tile_pool(name="gp", bufs=2))
        psum = ctx.enter_context(
            tc.tile_pool(name="psum", bufs=1, space=bass.MemorySpace.PSUM)
        )

        xs_t = const.tile([128, 2 * BSV], F32, tag="xs")
        nc.gpsimd.dma_start(xs_t[:], xs[:])
        fpk_t = const.tile([128, nbas * NRT * 2 * 128], FP8, tag="fpk")
        nc.gpsimd.dma_start(fpk_t[:], fpk[:])
        eb_t = const.tile([128, NRT], F32, tag="eb")
        nc.gpsimd.dma_start(eb_t[:], eb[:])
        whd_t = const.tile([128, NRT], F32R, tag="whd")
        nc.gpsimd.dma_start(whd_t[:], whd[:])

        lz = [psum.tile([128, BSV], F32, tag=f"lz{rt}", name=f"lz{rt}")
              for rt in range(NRT)]

        nmm = reps * nbas
        for rep in range(reps):
            xq_t = gp.tile([128, 2 * BSV], FP8, tag="xq")
            nc.vector.tensor_copy(xq_t[:], xs_t[:])
            g1_t = gp.tile([128, 2 * BSV], FP8, tag="g1")
            nc.scalar.activation(g1_t[:], xs_t[:], AF.Gelu)
            gts = [xq_t, g1_t]
            if nbas >= 3:
                g2_t = gp.tile([128, 2 * BSV], FP8, tag="g2")
                nc.scalar.activation(g2_t[:], xs_t[:], AF.Tanh)
                gts.append(g2_t)
            rhs = [t[:].rearrange("p (two b) -> p two b", two=2) for t in gts]
            for s in range(nbas):
                imm = rep * nbas + s
                for rt in range(NRT):
                    lhsT = fpk_t[
                        :, (s * NRT + rt) * 256 : (s * NRT + rt + 1) * 256
                    ].rearrange("p (two m) -> p two m", two=2)
                    nc.tensor.matmul(
                        lz[rt][:, :], lhsT, rhs[s],
                        start=(imm == 0), stop=(imm == nmm - 1),
                        perf_mode=mybir.MatmulPerfMode.DoubleRow,
                    )

        yp = psum.tile([1, BSV], F32, tag="yp")
        for rt in range(NRT):
            z_t = const.tile([128, BSV], F32R, tag=f"z{rt}")
            nc.scalar.activation(z_t[:], lz[rt][:], AF.Exp,
                                 bias=eb_t[:, rt : rt + 1], scale=expscale)
            nc.tensor.matmul(yp[:, :], whd_t[:, rt : rt + 1], z_t[:],
                             start=(rt == 0), stop=(rt == NRT - 1))
        y_sb = const.tile([1, BSV], F32, tag="ysb")
        nc.vector.tensor_copy(y_sb[:], yp[:])
        nc.sync.dma_start(y[:], y_sb[:])

    nc.compile()
    return nc


def _get_nc_v2(reps, tag, expscale, nbas=NBAS):
    key = ("v2", reps, tag, float(expscale), nbas)
    if key not in _cache:
        _cache[key] = _build_v2(reps, tag, expscale, nbas)
    return _cache[key]


# ======================================================================
# V2 host-side basis optimization + fit + packing
# ======================================================================


def _mask_const(inputs):
    mk = np.asarray(inputs["mask_logit"], dtype=np.float64)
    v = mk.reshape(-1)[0]
    return float(v) if np.all(mk == v) else None


def _prep_v2(inputs, nbas=NBAS):
    """Returns (in_maps, tag, hb, expscale) or None if not applicable."""
    mkv = _mask_const(inputs)
    if mkv is None:
        return None
    m = 1.0 / (1.0 + np.exp(-np.float64(mkv)))
    if not (1e-8 < m < 1 - 1e-8):
        return None

    x = np.asarray(inputs["x"], dtype=np.float64)
    sg = np.asarray(inputs["sign_param"], dtype=np.float64)
    th = np.asarray(inputs["th"], dtype=np.float64)
    lk = float(np.asarray(inputs["log_kappa"], dtype=np.float64).reshape(-1)[0])
    hwt = np.asarray(inputs["head_w"], dtype=np.float64).reshape(-1)
    hb = float(np.asarray(inputs["head_b"], dtype=np.float64).reshape(-1)[0])

    kappa = np.exp(lk)
    a = kappa * np.tanh(sg)                          # (R, D)
    if float(np.abs(a * th).max()) > 1e-12:
        # per-element bias b = a*th not supported by the shared-function
        # dictionary; leave to the NPHI-3 fallback (which handles biases)
        return None
    amax = float(np.abs(a).max())
    xmax = float(np.abs(x).max())
    if amax == 0.0:
        return None

    L = max(5.6, 1.05 * xmax)
    NX = 1601
    xg = np.linspace(-L, L, NX)
    hcnt, _ = np.histogram(x.reshape(-1), bins=NX,
                           range=(-L - L/(NX-1)/2, L + L/(NX-1)/2))
    wx = hcnt.astype(np.float64) + hcnt.max() * 1e-3
    wx /= wx.sum()

    rng = np.random.default_rng(0)
    af = a.reshape(-1)
    sel = rng.choice(af.size, min(V2_ASUB, af.size), replace=False)
    Psub = _phi64(af[sel][:, None] * xg[None, :], m)

    def fit_sub(Dm):
        Sdim = Dm.shape[1]
        G = (Dm * wx[:, None]).T @ Dm
        lam = np.zeros(Sdim)
        for s in range(1, Sdim):
            lam[s] = V2_LAM**2 * float((Dm[:, s]**2 * wx).sum())
        K2 = np.linalg.solve(G + np.diag(lam), (Dm * wx[:, None]).T)
        return Psub @ K2.T, G

    def opt_basis(fixed_cols, odd):
        g = _phi64(0.45 * amax * xg, m)
        g = g - (g * wx).sum()
        g /= np.sqrt((g**2 * wx).sum())
        if odd:
            g = 0.5 * (g - g[::-1])
        for _ in range(V2_NITER):
            cols = [fixed_cols[0]] + [
                _q8(c, _scale_exp(np.abs(c).max() + 1e-12))
                for c in fixed_cols[1:] + [g]]
            Dm = np.stack(cols, 1)
            C, _ = fit_sub(Dm)
            T = Psub - C[:, :-1] @ Dm[:, :-1].T
            c = C[:, -1]
            g = (c @ T) / (c @ c)
            if odd:
                g = 0.5 * (g - g[::-1])
            else:
                g = g - (g * wx).sum()
            r = np.sqrt((g**2 * wx).sum())
            g = np.clip(g / r, -V2_CAP, V2_CAP)
            if odd:
                g = 0.5 * (g - g[::-1])
            else:
                g = g - (g * wx).sum()
            g /= np.sqrt((g**2 * wx).sum())
        return g

    one = np.ones_like(xg)
    g1 = opt_basis([one, xg], odd=False)
    g2 = (opt_basis([one, xg, g1], odd=True) if nbas >= 3
          else np.zeros_like(xg))

    # quantized basis columns (exactly the device grid)
    e_x = 0
    e1 = _scale_exp(np.abs(g1).max())
    e2 = _scale_exp(np.abs(g2).max()) if nbas >= 3 else 0
    cols = [one, _q8(xg, e_x), _q8(g1, e1)]
    if nbas >= 3:
        cols.append(_q8(g2, e2))
    Dm = np.stack(cols, 1)
    S = Dm.shape[1]
    G = (Dm * wx[:, None]).T @ Dm
    lam = np.zeros(S)
    for s in range(1, S):
        lam[s] = V2_LAM**2 * float((Dm[:, s]**2 * wx).sum())
    K2 = np.linalg.solve(G + np.diag(lam), (Dm * wx[:, None]).T)

    C = np.empty((af.size, S))
    for i0 in range(0, af.size, 8192):
        P = _phi64(af[i0:i0 + 8192, None] * xg[None, :], m)
        C[i0:i0 + 8192] = P @ K2.T
    cmax = float(np.abs(C[:, 1:]).max())
    ce = _scale_exp(cmax)

    # coordinate-descent rounding to the fp8 grid
    bvec = C @ G
    Cq = C.copy()
    for s in range(1, S):
        Cq[:, s] = _q8(Cq[:, s], ce)
    for _ in range(5):
        for s in range(1, S):
            num = (bvec[:, s] - Cq @ G[:, s] + Cq[:, s] * G[s, s]) / G[s, s]
            Cq[:, s] = _q8(num, ce)
    num0 = bvec[:, 0] - Cq @ G[:, 0] + Cq[:, 0] * G[0, 0]
    Cq[:, 0] = num0 / G[0, 0]

    # residual gate: per-rule logz residual std (grid-weighted)
    res2 = np.empty(af.size)
    for i0 in range(0, af.size, 8192):
        P = _phi64(af[i0:i0 + 8192, None] * xg[None, :], m)
        Rm = P - Cq[i0:i0 + 8192] @ Dm.T
        res2[i0:i0 + 8192] = (Rm * Rm) @ wx
    err_r = np.sqrt(res2.reshape(R, D).sum(axis=1))
    if float(err_r.max()) > V2_GATE:
        return None

    # per-basis product scale folding: F_s = Cq_s * 2^(P - e_s) in fp8
    es = [e_x, e1, e2][:nbas]
    Pexp = min(es[s] + _scale_exp(
        float(np.abs(Cq[:, s + 1]).max()) + 1e-30) for s in range(nbas))
    expscale = float(2.0 ** (-Pexp))

    Cr = Cq.reshape(R, D, S)
    fpk_cols = np.empty((128, nbas * NRT * 2 * 128), dtype=NP8)
    for s in range(nbas):
        F = Cr[:, :, s + 1] * 2.0 ** (Pexp - es[s])       # (R, D)
        t4 = F.reshape(NRT, 128, 2, 128).transpose(3, 0, 2, 1)  # [p,rt,h,j]
        fpk_cols[:, s * NRT * 256 : (s + 1) * NRT * 256] = (
            t4.reshape(128, NRT * 256).astype(np.float32).astype(NP8))

    eb_arr = np.ascontiguousarray(
        Cr[:, :, 0].sum(axis=1).reshape(NRT, 128).T, dtype=np.float32)
    whd_arr = np.ascontiguousarray(
        hwt.reshape(NRT, 128).T, dtype=np.float32)

    # activation tables: gelu := g1 * 2^e1, tanh := g2 * 2^e2
    fg = lambda u: np.interp(u, xg, g1 * 2.0**e1)
    ft = lambda u: np.interp(u, xg, g2 * 2.0**e2)
    key_bytes = (np.float32(g1).tobytes() + np.float32(g2).tobytes()
                 + np.int64([e1, e2, Pexp, nbas]).tobytes())
    json_path, tag = _gen_act_tables_v2(fg, ft, float(L), key_bytes)

    xT = x.T                                          # (D, B)
    in_maps = []
    for c in range(NCORES):
        bsl = slice(c * BSV, (c + 1) * BSV)
        xs_arr = np.concatenate([xT[0:128, bsl], xT[128:256, bsl]], axis=1)
        in_maps.append({
            "xs": np.ascontiguousarray(xs_arr, dtype=np.float32),
            f"fpk8_{tag}": fpk_cols,
            "eb": eb_arr,
            "whd": whd_arr,
        })
    os.environ["BASS_ACT_ROOT_JSON_PATH"] = json_path
    return in_maps, tag, hb, expscale


# ======================================================================
# Dispatch
# ======================================================================


def _run(inputs, reps=1, **spmd_kwargs):
    prep2 = _prep_v2(inputs)
    if prep2 is not None:
        in_maps, tag, hb, expscale = prep2
        nc = _get_nc_v2(reps, tag, expscale)
        res = run_bass_kernel_spmd(nc, in_maps, core_ids=list(range(NCORES)),
                                   **spmd_kwargs)
        y = np.concatenate(
            [np.asarray(r["y"][0], dtype=np.float32) for r in res.results]
        ) + np.float32(hb)
        return y.astype(np.float32), res

    prep = _prep_lr(inputs)
    if prep is not None:
        in_maps, alphas, tag, hb = prep
        nc = _get_nc_lr(reps, alphas, tag)
        res = run_bass_kernel_spmd(nc, in_maps, core_ids=list(range(NCORES)),
                                   **spmd_kwargs)
        y = np.empty(B, dtype=np.float32)
        for gb in range(GB):
            acc = np.zeros(BS, dtype=np.float32)
            for gr in range(GR):
                acc += np.asarray(res.results[gr * GB + gb]["y"][0],
                                  dtype=np.float32)
            y[gb * BS : (gb + 1) * BS] = acc + np.float32(hb)
        return y, res

    os.environ.pop("BASS_ACT_ROOT_JSON_PATH", None)
    nc = _get_nc(reps)
    in_maps = _make_in_maps(inputs)
    res = run_bass_kernel_spmd(nc, in_maps, core_ids=list(range(NCORES)),
                               **spmd_kwargs)
    hb = np.asarray(inputs["head_b"], dtype=np.float32).reshape(-1)[0]
    y = np.sum([r["y"][0] for r in res.results], axis=0, dtype=np.float32) + hb
    return y.astype(np.float32), res


def kernel(**inputs) -> np.ndarray:
    y, _ = _run(inputs)
    return y

